# revision 9
# baseline (speedup 1.0000x reference)
"""Planesweep groupwise correlation on 8 TRN2 NeuronCores.

Sharding: core k handles n = k//4 and s in [8*(k%4), 8*(k%4)+8).
Per core (fixed n):
  setup: normalize feat_src over C; build a DRAM patch table where cell
    (yc, xc) = the 2x2 bilinear neighborhood (4 blocks x 64 bf16 channels,
    512B/cell, zero guards); stage a pixel-major bf16 copy of feat_ref.
  per (n,s): grid -> x0/y0/weights/mask; idx = (y0+1)*162 + (x0+1);
    fold wmask/||ref|| into the 4 corner weights; wrap idxs via a DRAM
    round trip; dma_gather one 512B patch per output pixel (4 SWDGE queues);
    DVE: prod = patch * ref4 (in place), XY-reduce over (gc) -> pc,
    corner-weight + reduce over k -> corr [h, w, 8]; outputs pixel-major,
    host reorders to [N, G, S, H, W].
"""
import sys
import numpy as np

if "/opt/trn_rl_repo" not in sys.path:
    sys.path.insert(0, "/opt/trn_rl_repo")

from contextlib import ExitStack
from concourse import bacc, tile, bass_utils, mybir
from concourse.masks import make_identity

F32 = mybir.dt.float32
BF16 = mybir.dt.bfloat16
I16 = mybir.dt.int16
I32 = mybir.dt.int32
OP = mybir.AluOpType
ACT = mybir.ActivationFunctionType
AX = mybir.AxisListType

H, W, C, G = 128, 160, 64, 8
HW = H * W                 # 20480
SL = 8                     # s per core
CELLW = W + 2              # 162
CELLS = (H + 1) * CELLW    # 20898
NQ = 4
QW = 40                    # w-columns per quarter
QIDX = 128 * QW            # 5120 idxs per gather

_CACHE = {}


def _emit(nc, tc, ctx, src_in, ref_in, grids_in, corr_o, wmask_o,
          tbl, refpix, ssq_src_d, ssq_ref_d, idxscr):
    psum = ctx.enter_context(tc.tile_pool(name="psA", bufs=3, space="PSUM"))
    psumB = ctx.enter_context(tc.tile_pool(name="psB", bufs=2, space="PSUM"))
    psumS = ctx.enter_context(tc.tile_pool(name="psS", bufs=2, space="PSUM"))

    tblv = tbl[:].rearrange("(y x) (k c) -> y x k c", y=H + 1, x=CELLW, k=4)

    # ---------------- setup ----------------
    with tc.tile_pool(name="setup", bufs=1) as sp:
        ones2 = sp.tile([128, 2], F32)
        nc.gpsimd.memset(ones2[:], 0.0)
        nc.gpsimd.memset(ones2[0:64, 0:1], 1.0)
        nc.gpsimd.memset(ones2[64:128, 1:2], 1.0)
        ident = sp.tile([128, 64], BF16)
        make_identity(nc, ident[0:64, :])
        make_identity(nc, ident[64:128, :])
        zt = sp.tile([128, 256], BF16)
        nc.gpsimd.memset(zt[:], 0.0)
        ztv = zt[:].rearrange("p (k c) -> p k c", k=4)

        # guard zeros first (interior writes below overwrite valid blocks)
        nc.sync.dma_start(tblv[0, 0:128], ztv)
        nc.sync.dma_start(tblv[0, 128:162], ztv[0:34])
        nc.sync.dma_start(tblv[128, 0:128], ztv)
        nc.sync.dma_start(tblv[128, 128:162], ztv[0:34])
        nc.sync.dma_start(tblv[1:128, 0], ztv[0:127])
        nc.sync.dma_start(tblv[1:128, 161], ztv[0:127])

        halves = {}
        for name, din, dssq in (("src", src_in, ssq_src_d), ("ref", ref_in, ssq_ref_d)):
            big = sp.tile([128, HW // 2], F32, tag="big1")
            nc.sync.dma_start(big[:], din[:])
            sq = sp.tile([128, HW // 2], F32, tag="big2")
            nc.vector.tensor_tensor(sq[:], big[:], big[:], OP.mult)
            ssq = sp.tile([2, HW // 2], F32, tag="norm")
            for t in range(HW // 2 // 512):
                ps = psumS.tile([2, 512], F32, tag="pssq")
                nc.tensor.matmul(ps[:], ones2[:], sq[:, 512 * t:512 * (t + 1)],
                                 start=True, stop=True)
                nc.scalar.activation(ssq[:, 512 * t:512 * (t + 1)], ps[:], ACT.Identity)
            nc.sync.dma_start(dssq.rearrange("(a b) -> a b", a=2), ssq[:])
            hb = sp.tile([128, HW // 2], BF16, tag=f"half_{name}")
            nc.vector.tensor_copy(hb[:], big[:])
            halves[name] = hb

        # src rnorm in [w, h] staging layout: sqrt+eps+recip after reload
        def _rn_from(dram_ap, out_tile):
            t1 = sp.tile(list(out_tile.shape), F32, tag="rns_t")
            nc.sync.dma_start(t1[:], dram_ap)
            t2 = sp.tile(list(out_tile.shape), F32, tag="rns_t2")
            nc.scalar.activation(t2[:], t1[:], ACT.Sqrt)
            nc.vector.tensor_scalar(t1[:], t2[:], 1e-9, None, OP.add)
            nc.vector.reciprocal(out_tile[:], t1[:])

        rnsA = sp.tile([128, H], F32)
        _rn_from(ssq_src_d.rearrange("(h w) -> w h", h=H)[0:128, :], rnsA)
        rnsB = sp.tile([32, H], F32)
        _rn_from(ssq_src_d.rearrange("(h w) -> w h", h=H)[128:160, :], rnsB)

        # staging transposes to pixel-major [w, h, c]; src scaled, ref raw
        for name in ("src", "ref"):
            feat = halves[name]
            stA = sp.tile([128, H, C], BF16, tag="stA")
            stB = sp.tile([32, H, C], BF16, tag="stB")
            for h0 in range(0, H, 8):
                pa = psum.tile([128, 512], BF16, tag="pta")
                pb = psumB.tile([32, 512], BF16, tag="ptb")
                for j in range(8):
                    h = h0 + j
                    pr = slice(0, 64) if h < 64 else slice(64, 128)
                    col = (h % 64) * W
                    nc.tensor.transpose(pa[:, 64 * j:64 * (j + 1)],
                                        feat[pr, col:col + 128], ident[pr, :])
                    nc.tensor.transpose(pb[:, 64 * j:64 * (j + 1)],
                                        feat[pr, col + 128:col + 160], ident[pr, :])
                pav = pa[:].rearrange("p (j c) -> p j c", j=8)
                pbv = pb[:].rearrange("p (j c) -> p j c", j=8)
                if name == "src":
                    nc.vector.tensor_tensor(
                        stA[:, h0:h0 + 8, :], pav,
                        rnsA[:, h0:h0 + 8].unsqueeze(2).broadcast_to([128, 8, C]),
                        OP.mult)
                    nc.vector.tensor_tensor(
                        stB[:, h0:h0 + 8, :], pbv,
                        rnsB[:, h0:h0 + 8].unsqueeze(2).broadcast_to([32, 8, C]),
                        OP.mult)
                else:
                    nc.vector.tensor_copy(stA[:, h0:h0 + 8, :], pav)
                    nc.vector.tensor_copy(stB[:, h0:h0 + 8, :], pbv)

            if name == "src":
                for dy in (0, 1):
                    for dx in (0, 1):
                        k = dy * 2 + dx
                        dst = tblv[1 - dy:129 - dy, 1 - dx:161 - dx, k, :]
                        dstT = dst.transpose([1, 0, 2])   # [x(=w), y(=h), c]
                        nc.sync.dma_start(dstT[0:128], stA[:])
                        nc.sync.dma_start(dstT[128:160], stB[:])
            else:
                rpv = refpix.rearrange("(h w) c -> w h c", h=H)
                nc.sync.dma_start(rpv[0:128], stA[:])
                nc.sync.dma_start(rpv[128:160], stB[:])

    # ---------------- persistent tiles (after setup space is released) ---
    perm = ctx.enter_context(tc.tile_pool(name="perm", bufs=1))
    rnrefN = perm.tile([128, W], F32)
    ref4 = perm.tile([128, W, 4, C], BF16)
    with tc.tile_pool(name="setup2", bufs=1) as sp2:
        t1 = sp2.tile([128, W], F32, tag="pt1")
        nc.sync.dma_start(t1[:], ssq_ref_d.rearrange("(h w) -> h w", h=H))
        t2 = sp2.tile([128, W], F32, tag="pt2")
        nc.scalar.activation(t2[:], t1[:], ACT.Sqrt)
        nc.vector.tensor_scalar(t1[:], t2[:], 1e-9, None, OP.add)
        nc.vector.reciprocal(rnrefN[:], t1[:])
        rtmp = sp2.tile([128, W, C], BF16, tag="rtmp")
        nc.sync.dma_start(rtmp[:], refpix.rearrange("(h w) c -> h w c", h=H))
        for k in range(4):
            nc.vector.tensor_copy(ref4[:, :, k, :], rtmp[:])

    # ---------------- main loop ----------------
    gpool = ctx.enter_context(tc.tile_pool(name="gath", bufs=2))
    wpool = ctx.enter_context(tc.tile_pool(name="wrk", bufs=2))
    spool = ctx.enter_context(tc.tile_pool(name="small", bufs=2))

    for sl in range(SL):
        gxy = {}
        for i, nm in ((0, "x"), (1, "y")):
            g = spool.tile([128, W], F32, tag=f"g{nm}")
            nc.sync.dma_start(g[:], grids_in[sl, i])
            gxy[nm] = g

        # floor / frac / valid per axis
        ax = {}
        for nm, lim in (("x", 158.0), ("y", 126.0)):
            i_ = spool.tile([128, W], F32, tag="txA")
            nc.vector.tensor_scalar(i_[:], gxy[nm][:], -0.5, None, OP.add)
            ti = spool.tile([128, W], I32, tag="txB")
            nc.vector.tensor_copy(ti[:], i_[:])
            tf = spool.tile([128, W], F32, tag="txC")
            nc.vector.tensor_copy(tf[:], ti[:])
            lt = spool.tile([128, W], F32, tag="txB2")
            nc.vector.tensor_tensor(lt[:], i_[:], tf[:], OP.is_lt)
            x0 = spool.tile([128, W], F32, tag=f"x0{nm}")
            nc.vector.tensor_tensor(x0[:], tf[:], lt[:], OP.subtract)
            w1 = spool.tile([128, W], F32, tag=f"w1{nm}")
            nc.vector.tensor_tensor(w1[:], i_[:], x0[:], OP.subtract)
            w0 = spool.tile([128, W], F32, tag=f"w0{nm}")
            nc.vector.tensor_scalar(w0[:], w1[:], -1.0, 1.0, OP.mult, OP.add)
            v0 = spool.tile([128, W], F32, tag=f"v0{nm}")
            nc.vector.tensor_scalar(v0[:], x0[:], 0.0, None, OP.is_ge)
            v1 = spool.tile([128, W], F32, tag=f"v1{nm}")
            nc.vector.tensor_scalar(v1[:], x0[:], lim, None, OP.is_le)
            ax[nm] = (x0, w0, w1, v0, v1)

        x0, wx0, wx1, vx0, vx1 = ax["x"]
        y0, wy0, wy1, vy0, vy1 = ax["y"]

        # idx = y0*162 + x0 + 163 -> int16
        idf = spool.tile([128, W], F32, tag="txA")
        nc.vector.tensor_scalar(idf[:], y0[:], 162.0, 163.0, OP.mult, OP.add)
        idf2 = spool.tile([128, W], F32, tag="txC")
        nc.vector.tensor_tensor(idf2[:], idf[:], x0[:], OP.add)
        idx16 = spool.tile([128, W], I16, tag="txB")
        nc.vector.tensor_copy(idx16[:], idf2[:])
        par = sl % 2
        nc.sync.dma_start(idxscr[par].rearrange("(w h) -> h w", h=H), idx16[:])

        # corner weights, mask, final scale
        wk = []
        for (wyy, vyy) in ((wy0, vy0), (wy1, vy1)):
            for (wxx, vxx) in ((wx0, vx0), (wx1, vx1)):
                w_ = spool.tile([128, W], F32, tag=f"w{len(wk)}")
                nc.vector.tensor_tensor(w_[:], wxx[:], wyy[:], OP.mult)
                v_ = spool.tile([128, W], F32, tag="vv")
                nc.vector.tensor_tensor(v_[:], vxx[:], vyy[:], OP.mult)
                wk.append((w_, v_))
        m = spool.tile([128, W], F32, tag="m")
        nc.vector.tensor_tensor(m[:], wk[0][0][:], wk[0][1][:], OP.mult)
        for (w_, v_) in wk[1:]:
            t_ = spool.tile([128, W], F32, tag="txC")
            nc.vector.tensor_tensor(t_[:], w_[:], v_[:], OP.mult)
            nc.vector.tensor_tensor(m[:], m[:], t_[:], OP.add)
        wm = spool.tile([128, W], F32, tag="txA")
        nc.vector.tensor_scalar(wm[:], m[:], 0.9999, None, OP.is_ge)
        nc.sync.dma_start(wmask_o[sl], wm[:])
        fs = spool.tile([128, W], F32, tag="txB2")
        nc.vector.tensor_tensor(fs[:], wm[:], rnrefN[:], OP.mult)
        wcat = spool.tile([128, W, 4], F32, tag="wcat")
        for k in range(4):
            nc.vector.tensor_tensor(wcat[:, :, k], wk[k][0][:], fs[:], OP.mult)

        corr = wpool.tile([128, W, G], F32, tag="corr")

        for q in range(4):
            qn = (sl + q) % NQ
            wr = gpool.tile([128, QIDX // 16], I16, tag="wr")
            base = QIDX * q
            for gg in (0, 1):
                nc.sync.dma_start(
                    wr[32 * qn + 16 * gg:32 * qn + 16 * (gg + 1), :],
                    idxscr[par][base:base + QIDX]
                    .rearrange("(s r) -> r s", r=16))
            gath = gpool.tile([128, QW, 256], BF16, tag="gath")
            nc.gpsimd.dma_gather(
                gath[:], tbl[:], wr[:], QIDX, QIDX,
                elem_size=256, elem_step=256,
                queue_num=qn, single_packet=False)

            # prod = gath * ref4 (in place), then XY-reduce over gc
            g4 = gath[:].rearrange("p w (k c) -> p w k c", k=4)
            nc.vector.tensor_tensor(g4, g4, ref4[:, QW * q:QW * (q + 1), :, :],
                                    OP.mult)
            pc1 = wpool.tile([128, QW * 4, G], F32, tag="pc1")
            gv = gath[:].rearrange("p w (k g a b) -> p (w k) g a b",
                                   k=4, g=G, a=2, b=4)
            nc.vector.tensor_reduce(pc1[:], gv, AX.XY, OP.add)
            # corner weighting (in place) and k-reduce
            pv = pc1[:].rearrange("p (w k) g -> p w k g", k=4)
            wv = (wcat[:, QW * q:QW * (q + 1), :].unsqueeze(3)
                  .broadcast_to([128, QW, 4, G]))
            nc.vector.tensor_tensor(pv, pv, wv, OP.mult)
            wt = pv.transpose([0, 1, 3, 2])   # [p, w, g, k]
            nc.vector.tensor_reduce(corr[:, QW * q:QW * (q + 1), :], wt,
                                    AX.X, OP.add)

        nc.sync.dma_start(corr_o[sl], corr[:])


def _build():
    nc = bacc.Bacc("TRN2", target_bir_lowering=False, debug=False,
                   num_devices=8, num_swdge_queues=NQ)
    src_in = nc.dram_tensor("src", [128, HW // 2], F32, kind="ExternalInput").ap()
    ref_in = nc.dram_tensor("ref", [128, HW // 2], F32, kind="ExternalInput").ap()
    grids_in = nc.dram_tensor("grids", [SL, 2, H, W], F32, kind="ExternalInput").ap()
    corr_o = nc.dram_tensor("corr_o", [SL, H, W, G], F32, kind="ExternalOutput").ap()
    wmask_o = nc.dram_tensor("wmask_o", [SL, H, W], F32, kind="ExternalOutput").ap()
    tbl = nc.dram_tensor("tbl", [CELLS, 256], BF16, kind="Internal").ap()
    refpix = nc.dram_tensor("refpix", [HW, C], BF16, kind="Internal").ap()
    ssq_src_d = nc.dram_tensor("ssq_src_d", [HW], F32, kind="Internal").ap()
    ssq_ref_d = nc.dram_tensor("ssq_ref_d", [HW], F32, kind="Internal").ap()
    idxscr = nc.dram_tensor("idxscr", [2, HW], I16, kind="Internal").ap()

    with tile.TileContext(nc) as tc:
        with ExitStack() as ctx:
            _emit(nc, tc, ctx, src_in, ref_in, grids_in, corr_o, wmask_o,
                  tbl, refpix, ssq_src_d, ssq_ref_d, idxscr)
    nc.compile()
    return nc


def get_nc():
    if "nc" not in _CACHE:
        _CACHE["nc"] = _build()
    return _CACHE["nc"]


def kernel(feat_ref, feat_src, grids, num_groups):
    assert int(num_groups) == G
    feat_ref = np.asarray(feat_ref, dtype=np.float32)
    feat_src = np.asarray(feat_src, dtype=np.float32)
    grids = np.asarray(grids, dtype=np.float32)
    N, S = feat_ref.shape[0], grids.shape[1]
    assert feat_ref.shape == (N, C, H, W) and S == 32

    nc = get_nc()
    in_maps = []
    for k in range(8):
        n, s0 = k // 4, 8 * (k % 4)
        f2 = lambda x: np.vstack([x[:, :HW // 2], x[:, HW // 2:]]).copy()
        in_maps.append({
            "src": f2(feat_src[n].reshape(C, HW)),
            "ref": f2(feat_ref[n].reshape(C, HW)),
            "grids": np.ascontiguousarray(grids[n, s0:s0 + SL]),
        })
    res = bass_utils.run_bass_kernel_spmd(nc, in_maps, core_ids=list(range(8)))

    corr = np.empty((N, G, S, H, W), dtype=np.float32)
    wmask = np.empty((N, S, H, W), dtype=np.float32)
    for k in range(8):
        n, s0 = k // 4, 8 * (k % 4)
        co = res.results[k]["corr_o"]          # [SL, H, W, G]
        corr[n, :, s0:s0 + SL] = co.transpose(3, 0, 1, 2)
        wmask[n, s0:s0 + SL] = res.results[k]["wmask_o"]
    return corr, wmask
